# revision 21
# baseline (speedup 1.0000x reference)
"""Bass/Tile TRN2 kernel for per-token multi-head attention over heads.

Reference computation (per token t):
  qkv = x @ w_qkv + b_qkv                  # [t, 3072]
  q/k/v[h, d] = qkv[h*192 + {0,64,128} + d]
  scores[h, g] = q[h] . k[g] / 8
  attn = softmax(scores, axis=g)
  out[h, d] = sum_g attn[h, g] v[g, d]
  y = out.reshape(1024) @ w_out + b_out
Sharding: tokens (B*S = 32768) split evenly over 8 cores; weights replicated.

Layout notes:
  - qkv computed transposed ([f x t], bf16) so per-head 64-row slabs are
    clean partition ranges; psum half-drains write straight into the
    q/k/v packs (no separate repack pass).
  - w_qkv columns are permuted host-side so each [128, 512] psum tile
    holds 4 same-kind slabs (2 f-tiles side by side); each psum then
    drains with 2 wide [64, 2, 256] ops instead of 4 narrow ones.
  - 8-token groups batch 16x16 per-token head-attention as 128x128
    block-diagonal matmuls; quads of groups share psum tiles and quad
    PAIRS share the back-transpose psum (psN8), letting the psN->outtok
    scatter run as 4 wide ops per pair instead of 16 narrow ones.
  - attention stages are software-pipelined inside a pair: scores+exp
    for both quads issue first, then V transposes, with next-iteration
    QKV matmuls as PE filler while exp/AV dependencies resolve. QKV
    drains are deferred to the slot end so the DVE FIFO never blocks
    the attention-critical rec/onorm ops.
  - x transposed by DMA-transpose, prefetched two iterations ahead.
  - biases in this problem are always zero: the no-bias build drops the
    bias adds entirely (drains become plain copies, proj loses its
    ones-row bias matmul). A bias-capable build is kept as fallback.
"""

import numpy as np
import ml_dtypes

H, DH = 16, 64
E = 1024
F3 = 3072
B, S = 4, 8192
N_CORES = 8
TOKS = (B * S) // N_CORES  # 4096 tokens per core
T = 256                    # tokens per unrolled iteration
NG = T // 8                # 8-token groups per iteration

NEG = -1.0e9


def _slot_head(s):
    # parity-major: slots 0-7 = even heads, 8-15 = odd heads
    return 2 * (s % 8) + s // 8


def _qkv_perm():
    """Column permutation of w_qkv: perm[nf] = original f.

    New layout: 12 psum-pair tiles m (kind = m//4: q,k,v; mm = m%4), each
    covering f-columns [(2m)*128, (2m+2)*128). Within tile m the slab at
    (jh, hh) (jh = which 128-col f-tile, hh = which 64-row half) holds
    head-slot s = 4*mm + 2*hh + jh.
    """
    perm = np.empty(F3, np.int64)
    for m in range(12):
        kind, mm = divmod(m, 4)
        for jh in range(2):
            for hh in range(2):
                s = 4 * mm + 2 * hh + jh
                h = _slot_head(s)
                nf0 = (2 * m + jh) * 128 + hh * 64
                f0 = h * 192 + kind * 64
                perm[nf0 : nf0 + 64] = np.arange(f0, f0 + 64)
    return perm


def build(toks_per_core=TOKS, with_bias=False):
    from concourse.bacc import Bacc
    import concourse.mybir as mybir
    from concourse.tile import TileContext
    from concourse.bass import ds

    f32 = mybir.dt.float32
    bf16 = mybir.dt.bfloat16
    niter = toks_per_core // T

    nc = Bacc("TRN2")
    x_d = nc.dram_tensor("x", [toks_per_core, E], bf16, kind="ExternalInput")
    wqkv_d = nc.dram_tensor("w_qkv", [E, F3], bf16, kind="ExternalInput")
    bqkv_d = nc.dram_tensor("b_qkv", [128, F3 // 128], f32, kind="ExternalInput")
    wout_d = nc.dram_tensor("w_out", [E, E], bf16, kind="ExternalInput")
    bout_d = nc.dram_tensor("b_out", [1, E], bf16, kind="ExternalInput")
    out_d = nc.dram_tensor("out", [toks_per_core, E], f32, kind="ExternalOutput")

    # mask fused into the scores matmul: extra contraction rows 64..71.
    # km[r, thi, s, tlo] = -sqrt(1e9) if tlo == r else 0
    # qm[r, s, t]        = +sqrt(1e9) if t % 8 != r else 0
    # => sum_r km.qm = -1e9 on t != t' pairs, exactly 0 on valid pairs
    SQ = np.float32(31623.0)
    tlo = np.arange(8)
    km_np = np.zeros((8, T // 8, H, 8), np.float32)
    km_np[tlo, :, :, tlo] = -SQ
    t_arr = np.arange(T)
    qm_np = np.where(
        (t_arr[None, None, :] % 8) != tlo[:, None, None], SQ, 0.0
    ).astype(np.float32)
    qm_np = np.broadcast_to(qm_np, (8, H, T)).copy()
    km_c = nc.inline_tensor(
        km_np.reshape(8, -1).astype(ml_dtypes.bfloat16), name="km_c"
    )
    qm_c = nc.inline_tensor(
        qm_np.reshape(8, -1).astype(ml_dtypes.bfloat16), name="qm_c"
    )
    identb_c = nc.inline_tensor(
        np.eye(128, dtype=ml_dtypes.bfloat16), name="identb_c"
    )
    ones_c = nc.inline_tensor(np.ones((1, 128), ml_dtypes.bfloat16), name="ones_c")

    with TileContext(nc) as tc:
        with (
            tc.tile_pool(name="persist", bufs=1) as pp,
            tc.tile_pool(name="xtp", bufs=3) as xtp,
            tc.tile_pool(name="attnsb", bufs=6) as attnsb,
            tc.tile_pool(name="outtokp", bufs=2) as outtokp,
            tc.tile_pool(name="outfp", bufs=3) as outfp,
            tc.tile_pool(name="psbig", bufs=4, space="PSUM") as psbig,
            tc.tile_pool(name="psattn", bufs=4, space="PSUM") as psattn,
        ):
            # ---- resident weights / constants ----
            # per-e-chunk weight tiles so the first matmuls don't wait on
            # the full 6 MB weight DMA
            wqkv_r = wqkv_d.rearrange("(ko kp) f -> kp ko f", kp=128)
            w_sb = []
            for e in range(8):
                w_e = pp.tile([128, F3], bf16, name=f"w_sb{e}")
                w_sb.append(w_e)
            wout_r = wout_d.rearrange("(ko kp) f -> kp ko f", kp=128)
            wout_sb = []
            for k2 in range(8):
                wo_e = pp.tile([128, E], bf16, name=f"wout_sb{k2}")
                wout_sb.append(wo_e)
            idb_sb = pp.tile([128, 128], bf16)
            if with_bias:
                bqkv_sb = pp.tile([128, F3 // 128], f32)
                nc.sync.dma_start(bqkv_sb, bqkv_d[:])
                bout_sb = pp.tile([1, E], bf16)
                nc.sync.dma_start(bout_sb, bout_d[:])
                ones_sb = pp.tile([1, 128], bf16)
                nc.sync.dma_start(ones_sb, ones_c[:])

            # double-buffered packs; mask rows are persistent constants
            qpacks, kpacks, vpacks = [], [], []
            for sidx in range(2):
                qpack = pp.tile([72, H, T], bf16, name=f"qpack{sidx}")
                kpack = pp.tile([72, T // 8, H, 8], bf16, name=f"kpack{sidx}")
                vpack = pp.tile([65, T // 8, H, 8], bf16, name=f"vpack{sidx}")
                nc.gpsimd.memset(vpack[64:65, :, :, :], 1.0)
                qpacks.append(qpack)
                kpacks.append(kpack)
                vpacks.append(vpack)

            def emit_xt(it, xt_sb, e):
                t0 = it * T
                nc.sync.dma_start_transpose(
                    xt_sb[:, e, :],
                    x_d[ds(t0, T), ds(e * 128, 128)],
                )

            # parity-major head slot (bias path only)
            def hslot(h):
                return (h % 2) * 8 + h // 2

            def emit_qkv_mm(it, xt_sb, m):
                # matmuls for pair-tile m; drain deferred (no-bias path)
                ps = psbig.tile([128, 512], f32, tag="ps_big")
                for jh in range(2):
                    dst = ps[:, ds(jh * 256, 256)]
                    for e in range(8):
                        nc.tensor.matmul(
                            dst,
                            w_sb[e][:, ds((2 * m + jh) * 128, 128)],
                            xt_sb[:, e, :],
                            start=(e == 0),
                            stop=(e == 7),
                        )
                return ps

            def emit_qkv_drain(it, m, ps):
                sidx = it % 2
                kind, mm = divmod(m, 4)
                for hh in range(2):
                    s0 = 4 * mm + 2 * hh
                    src = ps[ds(hh * 64, 64), :]
                    if kind == 0:
                        dst = qpacks[sidx][:64, ds(s0, 2), :]
                        nc.scalar.copy(
                            out=dst, in_=src.rearrange("p (j t) -> p j t", j=2)
                        )
                    else:
                        pk = kpacks[sidx] if kind == 1 else vpacks[sidx]
                        dst = pk[:64, :, ds(s0, 2), :]
                        src_r = src.rearrange("p (j g b) -> p g j b", j=2, b=8)
                        if kind == 1 and hh == 0:
                            # shift a quarter of the kv drains to ACT to
                            # balance the two PSUM-capable engines
                            nc.scalar.copy(out=dst, in_=src_r)
                        else:
                            nc.vector.tensor_copy(out=dst, in_=src_r)

            def emit_qkv_ftile_bias(it, xt_sb, j):
                # bias fallback: original unpermuted layout, 2 narrow drains
                sidx = it % 2
                psq_full = psbig.tile([128, 512], f32, tag="ps_big")
                psq = psq_full[:, :T]
                for e in range(8):
                    nc.tensor.matmul(
                        psq,
                        w_sb[e][:, ds(j * 128, 128)],
                        xt_sb[:, e, :],
                        start=(e == 0),
                        stop=(e == 7),
                    )
                for half in range(2):
                    frow = j * 128 + half * 64
                    h, rem = divmod(frow, 192)
                    which = rem // 64
                    sl = hslot(h)
                    if which == 0:
                        dst = qpacks[sidx][:64, sl, :]
                    elif which == 1:
                        dst = kpacks[sidx][:64, :, sl, :]
                    else:
                        dst = vpacks[sidx][:64, :, sl, :]
                    src = psq[half * 64 : half * 64 + 64, :]
                    bias_ap = bqkv_sb[half * 64 : half * 64 + 64, j : j + 1]
                    if which == 0:
                        nc.scalar.activation(
                            dst,
                            src,
                            mybir.ActivationFunctionType.Identity,
                            bias=bias_ap,
                            scale=1.0,
                        )
                    else:
                        nc.vector.scalar_tensor_tensor(
                            out=dst,
                            in0=src.rearrange("p (a b) -> p a b", b=8),
                            scalar=1.0,
                            in1=bias_ap[:, :, None].to_broadcast(
                                (64, T // 8, 8)
                            ),
                            op0=mybir.AluOpType.mult,
                            op1=mybir.AluOpType.add,
                        )

            def emit_attn_pair(it, outtok, q2, fillers, mid_work=None,
                               late_work=None):
                # two quads (8 groups, 64 tokens); `fillers` emit next-iter
                # QKV matmuls between dependency-bound attention stages
                sidx = it % 2
                qpack, kpack, vpack = qpacks[sidx], kpacks[sidx], vpacks[sidx]
                fill = list(fillers)
                pend = []

                def run_filler():
                    if fill:
                        pend.append(fill.pop(0)())

                # stage 1: scores + exp for both halves
                psS = []
                for half in range(2):
                    gs4 = [8 * q2 + 4 * half + i for i in range(4)]
                    psS4 = psattn.tile([128, 4, 128], f32, tag="ps_attn")
                    for i, g in enumerate(gs4):
                        nc.tensor.matmul(
                            psS4[:, i, :],
                            kpack[:, g, :, :].rearrange("p a b -> p (a b)"),
                            qpack[:, :, ds(g * 8, 8)],
                            start=True,
                            stop=True,
                        )
                    expS4 = attnsb.tile([128, 512], bf16, tag="expS")
                    nc.scalar.activation(
                        expS4[:],
                        psS4.rearrange("p a b -> p (a b)"),
                        mybir.ActivationFunctionType.Exp,
                        bias=0.0,
                        scale=0.125,
                    )
                    psS.append(expS4)
                # stage 2: V transposes for both halves, one wide copy
                psV8 = psattn.tile([128, 8, 66], bf16, tag="ps_attn")
                for k in range(8):
                    g = 8 * q2 + k
                    nc.tensor.transpose(
                        psV8[:, k, :65],
                        vpack[:, g, :, :].rearrange("p a b -> p (a b)"),
                        idb_sb[:65, :65],
                    )
                vt8_sb = attnsb.tile([128, 8, 65], bf16, tag="vt")
                nc.vector.tensor_copy(out=vt8_sb[:], in_=psV8[:, :, :65])
                if mid_work is not None:
                    mid_work()
                if late_work is not None:
                    late_work()
                run_filler()
                run_filler()
                # stage 3: AV + normalize per half
                onorm8 = attnsb.tile([128, 8, 64], bf16, tag="onorm")
                rec8 = attnsb.tile([128, 8], f32, tag="rec")
                for half in range(2):
                    expS4 = psS[half]
                    psAV4 = psattn.tile([128, 4, 65], f32, tag="ps_attn")
                    for i in range(4):
                        nc.tensor.matmul(
                            psAV4[:, i, :],
                            expS4[:, ds(i * 128, 128)],
                            vt8_sb[:, half * 4 + i, :],
                            start=True,
                            stop=True,
                        )
                    nc.vector.reciprocal(
                        rec8[:, ds(half * 4, 4)], psAV4[:, :, 64]
                    )
                    nc.vector.tensor_tensor(
                        onorm8[:, ds(half * 4, 4), :],
                        psAV4[:, :, 0:64],
                        rec8[:, ds(half * 4, 4), None].to_broadcast(
                            (128, 4, 64)
                        ),
                        mybir.AluOpType.mult,
                    )
                    if half == 1:
                        run_filler()
                # stage 4: back-transpose both quads into one psum tile,
                # then 4 wide scatters into outtok
                psN8 = psattn.tile([128, 4, 128], bf16, tag="ps_attn")
                for p2 in range(4):
                    nc.tensor.transpose(
                        psN8[:, p2, :],
                        onorm8[:, 2 * p2 : 2 * p2 + 2, :].rearrange(
                            "p a b -> p (a b)"
                        ),
                        idb_sb[:],
                    )
                otv = outtok.rearrange(
                    "p a (q pp i b) -> p q i a pp b", q=4, pp=4, i=2, b=8
                )
                for i in range(2):
                    for par in range(2):
                        src = psN8[
                            i * 64 : i * 64 + 64, :, ds(par * 64, 64)
                        ].rearrange("p pp (a b) -> p a pp b", b=8)
                        dst = otv[par * 64 : par * 64 + 64, q2, i, :, :, :]
                        nc.vector.tensor_copy(out=dst, in_=src)
                run_filler()
                while fill:
                    run_filler()
                return pend

            def emit_proj(it, outtok, jm):
                t0 = it * T
                for nh in range(2):
                    psO = psbig.tile([128, 512], f32, tag="ps_big")
                    for k2 in range(8):
                        last = (k2 == 7) and not with_bias
                        nc.tensor.matmul(
                            psO,
                            outtok[:, k2, ds(jm * 128, 128)],
                            wout_sb[k2][:, ds(nh * 512, 512)],
                            start=(k2 == 0),
                            stop=last,
                        )
                    if with_bias:
                        nc.tensor.matmul(
                            psO,
                            ones_sb[:, :],
                            bout_sb[:, ds(nh * 512, 512)],
                            start=False,
                            stop=True,
                        )
                    outf = outfp.tile([128, 512], f32, tag="outf")
                    nc.scalar.activation(
                        outf[:], psO, mybir.ActivationFunctionType.Copy
                    )
                    nc.sync.dma_start(
                        out_d[ds(t0 + jm * 128, 128), ds(nh * 512, 512)],
                        outf[:],
                    )

            def make_filler(nxt, xt_sb, m):
                if with_bias:
                    def fb():
                        emit_qkv_ftile_bias(nxt, xt_sb, 2 * m)
                        emit_qkv_ftile_bias(nxt, xt_sb, 2 * m + 1)
                        return None
                    return fb

                def f():
                    ps = emit_qkv_mm(nxt, xt_sb, m)
                    return (m, ps)
                return f

            # q+k pairs all drain by slot 2 (next-iter scores need
            # qpack+kpack first); v pairs last (vtrans runs later)
            slot_ms = [[0, 4, 1], [5, 2, 6], [3, 7, 8], [9, 10, 11]]

            # ---- software-pipelined schedule ----
            xt_tiles = {}

            def issue_xt(i2, e0):
                if i2 < niter:
                    if i2 not in xt_tiles:
                        xt_tiles[i2] = xtp.tile(
                            [128, 8, T], bf16, name=f"xt{i2 % 3}", tag="xt"
                        )
                    for e in range(e0, e0 + 4):
                        emit_xt(i2, xt_tiles[i2], e)

            # prologue DMA order: iter-0 x transposes interleaved with the
            # QKV weight chunks (both gate the first matmuls), then iter-1
            # x, then attention constants, then the out-proj weights
            issue_xt(0, 0)
            for e in range(4):
                nc.sync.dma_start(w_sb[e], wqkv_r[:, e, :])
            issue_xt(0, 4)
            for e in range(4, 8):
                nc.sync.dma_start(w_sb[e], wqkv_r[:, e, :])
            issue_xt(1, 0)
            issue_xt(1, 4)
            nc.sync.dma_start(idb_sb, identb_c[:])
            for sidx in range(2):
                nc.sync.dma_start(
                    qpacks[sidx][64:72, :, :].rearrange("p a b -> p (a b)"),
                    qm_c[:],
                )
                nc.sync.dma_start(
                    kpacks[sidx][64:72, :, :, :].rearrange(
                        "p a b c -> p (a b c)"
                    ),
                    km_c[:],
                )
            for k2 in range(8):
                nc.sync.dma_start(wout_sb[k2], wout_r[:, k2, :])
            if with_bias:
                for m in range(12):
                    emit_qkv_ftile_bias(0, xt_tiles[0], 2 * m)
                    emit_qkv_ftile_bias(0, xt_tiles[0], 2 * m + 1)
            else:
                # first 4 pairs contract e=0..3 while the e=4..7 x
                # transposes and weight chunks are still in flight
                open_ps = []
                for m in range(4):
                    ps = psbig.tile([128, 512], f32, tag="ps_big")
                    dst = ps[:, ds(0, 256)]
                    for e in range(4):
                        nc.tensor.matmul(
                            dst,
                            w_sb[e][:, ds(2 * m * 128, 128)],
                            xt_tiles[0][:, e, :],
                            start=(e == 0),
                            stop=False,
                        )
                    open_ps.append(ps)
                for m in range(4):
                    ps = open_ps[m]
                    dst = ps[:, ds(0, 256)]
                    for e in range(4, 8):
                        nc.tensor.matmul(
                            dst,
                            w_sb[e][:, ds(2 * m * 128, 128)],
                            xt_tiles[0][:, e, :],
                            start=False,
                            stop=(e == 7),
                        )
                for m in range(4):
                    ps = open_ps[m]
                    dst = ps[:, ds(256, 256)]
                    for e in range(8):
                        nc.tensor.matmul(
                            dst,
                            w_sb[e][:, ds((2 * m + 1) * 128, 128)],
                            xt_tiles[0][:, e, :],
                            start=(e == 0),
                            stop=(e == 7),
                        )
                    emit_qkv_drain(0, m, ps)
                for m in range(4, 12):
                    ps = emit_qkv_mm(0, xt_tiles[0], m)
                    emit_qkv_drain(0, m, ps)

            prev = None  # (outtok, it) with jm=1 proj still pending
            for it in range(niter):
                nxt = it + 1
                outtok = outtokp.tile([128, 8, T], bf16, tag="outtok")
                for q2 in range(4):
                    if nxt < niter:
                        fillers = [
                            make_filler(nxt, xt_tiles[nxt], m)
                            for m in slot_ms[q2]
                        ]
                    else:
                        fillers = []
                    mid = None
                    if q2 == 0 and prev is not None:
                        po, pit = prev

                        def mid(po=po, pit=pit):
                            emit_proj(pit, po, 1)
                    elif q2 == 2:

                        def mid(it=it, outtok=outtok):
                            emit_proj(it, outtok, 0)

                    pend = emit_attn_pair(it, outtok, q2, fillers, mid)
                    if q2 == 0:
                        issue_xt(it + 2, 0)
                    elif q2 == 1:
                        issue_xt(it + 2, 4)
                    for item in pend:
                        if item is not None:
                            m, ps = item
                            emit_qkv_drain(nxt, m, ps)
                prev = (outtok, it)
                xt_tiles.pop(it, None)
            emit_proj(prev[1], prev[0], 1)
    nc.finalize()
    return nc


_cache = {}


def _get_nc(toks_per_core=TOKS, with_bias=False):
    key = (toks_per_core, with_bias)
    if key not in _cache:
        _cache[key] = build(toks_per_core, with_bias)
    return _cache[key]


def prep_inputs(
    x, w_qkv, b_qkv, w_out, b_out, toks_per_core=TOKS, n_cores=N_CORES,
    with_bias=False,
):
    """Shard tokens over cores; replicate (host-preprocessed) weights."""
    xf = np.ascontiguousarray(
        np.asarray(x, dtype=np.float32).astype(ml_dtypes.bfloat16)
    ).reshape(-1, E)
    wq_np = np.asarray(w_qkv, dtype=np.float32)
    if not with_bias:
        wq_np = wq_np[:, _qkv_perm()]
    wq = np.ascontiguousarray(wq_np.astype(ml_dtypes.bfloat16))
    bq = np.ascontiguousarray(
        np.asarray(b_qkv, dtype=np.float32).reshape(F3 // 128, 128).T
    )
    wo = np.ascontiguousarray(np.asarray(w_out).astype(ml_dtypes.bfloat16))
    bo = np.ascontiguousarray(
        np.asarray(b_out, dtype=np.float32).astype(ml_dtypes.bfloat16).reshape(1, E)
    )
    in_maps = []
    for c in range(n_cores):
        in_maps.append(
            {
                "x": np.ascontiguousarray(
                    xf[c * toks_per_core : (c + 1) * toks_per_core]
                ),
                "w_qkv": wq,
                "b_qkv": bq,
                "w_out": wo,
                "b_out": bo,
            }
        )
    return in_maps


def run(x, w_qkv, b_qkv, w_out, b_out, toks_per_core=TOKS, n_cores=N_CORES, **kw):
    from concourse import bass_utils

    with_bias = bool(
        np.any(np.asarray(b_qkv)) or np.any(np.asarray(b_out))
    )
    nc = _get_nc(toks_per_core, with_bias)
    in_maps = prep_inputs(
        x, w_qkv, b_qkv, w_out, b_out, toks_per_core, n_cores, with_bias
    )
    res = bass_utils.run_bass_kernel_spmd(
        nc, in_maps, core_ids=list(range(n_cores)), **kw
    )
    out = np.concatenate([r["out"] for r in res.results], axis=0)
    return out, res


def kernel(x, w_qkv, b_qkv, w_out, b_out):
    out, _ = run(x, w_qkv, b_qkv, w_out, b_out)
    return out.reshape(x.shape[0], x.shape[1], E)
